# revision 3
# baseline (speedup 1.0000x reference)
"""Trainium2 Bass kernel for nn_DenseExpert (MoE dense-expert gated blend).

Math (full problem, B=8192, E=8, U=512, D=512):
    h[b,e,u] = sum_d x[b,d] * alpha[e,u,d]
    r[b,u]   = sum_e g[b,e] * h[b,e,u] + sum_e g[b,e] * beta[e,u]

Strategy:
  - Data-parallel over batch B across 8 NeuronCores (1024 rows each);
    alpha/beta replicated. No collectives.
  - Matmul operands in bf16 (fp8 DoubleRow measured 3.3e-2 rel err —
    over the 2e-2 budget — so the 2x fp8 path is unusable; bf16 PE
    streaming 131k cols @2.4GHz = 54.6us is the per-core floor).
  - The gated reduction is pipelined across the two idle elementwise
    engines instead of serializing on the DVE (which cost the previous
    version ~50us of DVE busy time):
      ACT: t = h_e * g[:,e]  (per-partition scale, PSUM f32 -> SBUF bf16)
      DVE: acc += t          (all-bf16 tensor_tensor, 2x_1p mode)
    The bias sum_e g*beta is precomputed on the host and DMA-preloaded
    into the bf16 accumulator, so every expert takes the same two ops.
  - A short burst of warmup matmuls on a zeroed dummy tile ramps the PE
    clock out of its low p-state while the first input DMAs land.
  - Output is written bf16 (host upcasts); batch tiles run in three
    phases (m0-4, m5-6, m7) so output DMA overlaps compute.
"""

import numpy as np
from contextlib import ExitStack

try:
    import concourse.bass as bass
except ImportError:  # fallback if concourse isn't on the default path
    import sys

    sys.path.insert(0, "/opt/trn_rl_repo")
    import concourse.bass as bass
from concourse import bacc

import concourse.mybir as mybir
import concourse.tile as tile
from concourse.bass_utils import run_bass_kernel_spmd

B, E, U, D = 8192, 8, 512, 512
N_CORES = 8
BC = B // N_CORES  # 1024 batch rows per core
P = 128
M_TILES = BC // P  # 8 batch tiles per core
K_TILES = D // P  # 4 contraction chunks
F32 = mybir.dt.float32
BF16 = mybir.dt.bfloat16

_NC_CACHE = {}
last_results = None  # BassKernelResults of the most recent run (for test harness)

PHASES = [(0, 5), (5, 7), (7, 8)]
N_WARMUP = 6  # dummy matmuls to ramp the PE clock while first DMAs land


def _build_nc():
    nc = bacc.Bacc("TRN2", target_bir_lowering=False, debug=False)

    xT = nc.dram_tensor("xT", [D, BC], BF16, kind="ExternalInput").ap()
    g = nc.dram_tensor("g", [BC, E], F32, kind="ExternalInput").ap()
    biasT = nc.dram_tensor("biasT", [BC, U], BF16, kind="ExternalInput").ap()
    alphaT = nc.dram_tensor("alphaT", [E, D, U], BF16, kind="ExternalInput").ap()
    out = nc.dram_tensor("out", [BC, U], BF16, kind="ExternalOutput").ap()

    add = mybir.AluOpType.add
    Copy = mybir.ActivationFunctionType.Copy

    with tile.TileContext(nc) as tc, ExitStack() as ctx:
        sml_pool = ctx.enter_context(tc.tile_pool(name="sml", bufs=1))
        at_pool = ctx.enter_context(tc.tile_pool(name="at", bufs=E))
        acc_pool = ctx.enter_context(tc.tile_pool(name="acc", bufs=1))
        t_pool = ctx.enter_context(tc.tile_pool(name="tst", bufs=6))
        ps_pool = ctx.enter_context(tc.tile_pool(name="ps", bufs=8, space="PSUM"))

        # ---- PE warmup: ramp the clock on a zeroed dummy while DMAs land
        dummy = sml_pool.tile([P, U], BF16, tag="dummy", name="dummy")
        nc.gpsimd.memset(dummy[:], 0)
        ps_warm = ps_pool.tile([P, U], F32, tag="ps", name="ps_warm")
        for w in range(N_WARMUP):
            nc.tensor.matmul(
                ps_warm[:], dummy[:, :P], dummy[:], start=True, stop=True
            )

        # ---- DMA issue order tuned so expert 0 starts immediately and
        # expert 1's alpha (the one DMA-paced stall in the previous
        # version) arrives in time: x/alpha0 per k-chunk first with
        # alpha1 split in two right behind, g before the first blend,
        # bias phase A before the first accumulate.
        xts = []
        at0k = []
        a1_t = at_pool.tile([P, K_TILES, U], BF16, tag="at", name="at1")
        for k in range(K_TILES):
            x_t = sml_pool.tile([P, BC], BF16, tag=f"xt{k}", name=f"xt{k}")
            nc.sync.dma_start(x_t[:], xT[k * P : (k + 1) * P, :])
            xts.append(x_t)
            a_t = sml_pool.tile([P, U], BF16, tag=f"at0k{k}", name=f"at0k{k}")
            nc.sync.dma_start(a_t[:], alphaT[0, k * P : (k + 1) * P, :])
            at0k.append(a_t)
            if k == 0:
                # all of g in one tiny DMA: [128, m, e]
                g_t = sml_pool.tile([P, M_TILES, E], F32, tag="g", name="gt")
                nc.sync.dma_start(g_t[:], g.rearrange("(m p) e -> p m e", p=P))
                a1r = alphaT[1].rearrange("(k p) u -> p k u", p=P)
                nc.sync.dma_start(a1_t[:, :2, :], a1r[:, :2, :])
            if k == 1:
                nc.sync.dma_start(a1_t[:, 2:, :], a1r[:, 2:, :])

        # bias (host-precomputed g @ beta), phased so phase A arrives first
        bias_r = biasT.rearrange("(m p) u -> p m u", p=P)
        acc_t = acc_pool.tile([P, M_TILES, U], BF16, tag="acc", name="acc")
        for a, b in PHASES:
            nc.sync.dma_start(acc_t[:, a:b, :], bias_r[:, a:b, :])

        # alpha^T experts 2..: one DMA each: [128, k, u]
        ats = [None, a1_t]
        for e in range(2, E):
            a_t = at_pool.tile([P, K_TILES, U], BF16, tag="at", name=f"at{e}")
            nc.sync.dma_start(a_t[:], alphaT[e].rearrange("(k p) u -> p k u", p=P))
            ats.append(a_t)

        # ---- experts, phased over batch tiles so output writes overlap
        # compute ----
        out_r = out.rearrange("(m p) u -> p m u", p=P)
        for a, b in PHASES:
            for e in range(E):
                pes = {}
                for m in range(a, b):
                    pes[m] = ps_pool.tile([P, U], F32, tag="ps", name=f"pe{e}_{m}")
                for k in range(K_TILES):
                    rhs = at0k[k][:] if e == 0 else ats[e][:, k, :]
                    for m in range(a, b):
                        nc.tensor.matmul(
                            pes[m][:],
                            xts[k][:, bass.ts(m, P)],
                            rhs,
                            start=(k == 0),
                            stop=(k == K_TILES - 1),
                        )
                for m in range(a, b):
                    # ACT: t = h_e * g[:,e]  (PSUM f32 -> SBUF bf16)
                    t_t = t_pool.tile([P, U], BF16, tag="t", name=f"t{e}_{m}")
                    nc.scalar.activation(
                        t_t[:], pes[m][:], Copy, scale=g_t[:, m, e : e + 1]
                    )
                    # DVE: acc += t  (all-bf16 -> 2x mode)
                    nc.vector.tensor_tensor(
                        acc_t[:, m, :], acc_t[:, m, :], t_t[:], op=add
                    )
            nc.sync.dma_start(out_r[:, a:b, :], acc_t[:, a:b, :])

    nc.compile()
    return nc


def _get_nc():
    if "nc" not in _NC_CACHE:
        _NC_CACHE["nc"] = _build_nc()
    return _NC_CACHE["nc"]


def kernel(x, g, alpha, beta, _trace=False, _trace_kwargs=None):
    global last_results
    import ml_dtypes

    bf16 = ml_dtypes.bfloat16
    x = np.asarray(x, dtype=np.float32)
    g = np.ascontiguousarray(np.asarray(g, dtype=np.float32))
    alpha = np.asarray(alpha, dtype=np.float32)
    beta = np.ascontiguousarray(np.asarray(beta, dtype=np.float32))

    # [E, D, U] in bf16 for halved DMA traffic
    alphaT = np.ascontiguousarray(alpha.transpose(0, 2, 1).astype(bf16))
    xTb = np.ascontiguousarray(x.T.astype(bf16))  # [D, B]
    bias = (g @ beta).astype(bf16)  # [B, U]

    in_maps = []
    for c in range(N_CORES):
        sl = slice(c * BC, (c + 1) * BC)
        in_maps.append(
            {
                "xT": np.ascontiguousarray(xTb[:, sl]),  # [D, BC] bf16
                "g": g[sl],  # [BC, E] f32
                "biasT": np.ascontiguousarray(bias[sl]),  # [BC, U] bf16
                "alphaT": alphaT,  # [E, D, U] bf16 (replicated)
            }
        )

    nc = _get_nc()
    res = run_bass_kernel_spmd(
        nc,
        in_maps,
        list(range(N_CORES)),
        trace=_trace,
        **(_trace_kwargs or {}),
    )
    last_results = res
    return np.concatenate(
        [np.asarray(r["out"]).astype(np.float32) for r in res.results], axis=0
    )
